# revision 33
# baseline (speedup 1.0000x reference)
"""AngularDistribution Trainium2 kernel (8 NeuronCores, SPMD over (batch,atom) pairs).

Math (per pair p, triple n, offset r, filter f):
  rad[n,r]  = exp(-g*(S2[n] - 2*o_r*S1[n] + 3*o_r^2))   S1=rij+rik+rjk, S2=sum sq
  ang[n,f]  = 2*u^z (f<4, u=(1-ct)/2) or 2*v^z (f>=4), z in {1,2,4,8}
  cm[n]     = 2*(cos(pi*rij/10)*cos(pi*rik/10)*cos(pi*rjk/10))^2
  out[p,r*8+f] = sum_n rad[n,r] * pw[n,f] * cm[n]    (mask via host compaction)

v4 structure per core (64 pairs, nch*128 triples padded, sorted by rbar):
  - host compacts, SORTS each pair's triples by rbar=(S1/3), pre-transposes to
    [128, 64*nch] (chunk-major: chunk j = rbar quantile block), contiguous DMA
  - rad[n, r] is ~zero outside |o_r - rbar| < 0.65, so each chunk only needs a
    W0-wide offset window; chunks are grouped into 4 groups sharing an exact
    window [lo_g, lo_g+W0) -> exponent matmul streams nch*W0 cols (not nch*32)
  - tiny matmuls accumulate window-LOCAL into 4 aligned 32-row PSUM bands;
    a constant 0/1 matrix un-shifts bands to global r via one matmul per 16
    pairs
  - S1/S2 source tile transposed on the PE in f32r (half the f32 cost)
  - power/cutoff chain in bf16 (2x DVE modes); exponent path stays f32
"""

import os
import sys

sys.path.insert(0, "/opt/trn_rl_repo")

import numpy as np
from contextlib import ExitStack

GAMMA = 4.0
N_CORES = 8
PP = 64          # pairs per core (512 total / 8)
R = 32
F = 8
DELTA = 4.5 / 31  # offset grid spacing
RADI = 4.5        # window radius in grid units (= 0.65 distance)

_CACHE = {}
LAST_EXEC_NS = None


def _build(cfg):
    nch, W0, lo_of_chunk, gof, gfirst, glast = cfg
    import concourse.bass as bass
    import concourse.tile as tile
    from concourse import bacc, mybir

    f32 = mybir.dt.float32
    f32r = mybir.dt.float32r
    bf16 = mybir.dt.bfloat16
    Alu = mybir.AluOpType
    Act = mybir.ActivationFunctionType
    W = PP * nch          # global tile free size
    KR = 2 * nch + 1      # lhsT rows per pair (S1 rows, S2 rows, ones row)
    KRP = 32              # padded row stride: 4 pairs per 128-row block
    NV = PP * KRP
    NB = NV // 128        # 128-col blocks in src/ts (== PP//4)
    WTOT = nch * W0       # exponent grid cols per pair (windowed)
    SPC = 512             # psu column spacing per pair (bank-aligned)
    PI = float(np.pi)
    assert WTOT <= SPC

    nc = bacc.Bacc("TRN2", target_bir_lowering=False, debug=False,
                   num_devices=N_CORES)

    d_rij = nc.dram_tensor("rij", [128, W], f32, kind="ExternalInput")
    d_rik = nc.dram_tensor("rik", [128, W], f32, kind="ExternalInput")
    d_rjk = nc.dram_tensor("rjk", [128, W], f32, kind="ExternalInput")
    d_bd = nc.dram_tensor("bdiag", [128, WTOT], f32, kind="ExternalInput")
    d_sel = nc.dram_tensor("sel", [128, R], f32, kind="ExternalInput")
    d_id = nc.dram_tensor("ident", [128, 128], f32, kind="ExternalInput")
    d_out = nc.dram_tensor("out", [R, PP * F], f32, kind="ExternalOutput")

    with tile.TileContext(nc) as tc, ExitStack() as ctx:
        cpool = ctx.enter_context(tc.tile_pool(name="consts", bufs=1))
        gpool = ctx.enter_context(tc.tile_pool(name="glob", bufs=1))
        rpool = ctx.enter_context(tc.tile_pool(name="rad", bufs=16))
        pupool = ctx.enter_context(tc.tile_pool(name="psu", bufs=4, space="PSUM"))
        pcpool = ctx.enter_context(tc.tile_pool(name="pc", bufs=2, space="PSUM"))
        p2pool = ctx.enter_context(tc.tile_pool(name="ps2", bufs=2, space="PSUM"))

        bias0 = cpool.tile([128, 1], f32)
        nc.vector.memset(bias0[:], 0.0)
        bias_hpi = cpool.tile([128, 1], f32)
        nc.vector.memset(bias_hpi[:], PI / 2.0)

        # ---- input tiles, contiguous DMA in pair chunks (small first) ----
        rij_t = gpool.tile([128, W], f32)
        rik_t = gpool.tile([128, W], f32)
        rjk_t = gpool.tile([128, W], f32)
        PCHK = [(0, 8), (8, 32), (32, 64)]
        NCHK = len(PCHK)

        def csl(c):
            return slice(PCHK[c][0] * nch, PCHK[c][1] * nch)

        for c in range(NCHK):
            sl = csl(c)
            for dst, src in ((rij_t, d_rij), (rik_t, d_rik), (rjk_t, d_rjk)):
                nc.sync.dma_start(dst[:, sl], src.ap()[:, sl])
        bd_t = cpool.tile([128, WTOT], f32r)
        nc.sync.dma_start(bd_t[:], d_bd.ap().bitcast(f32r))
        sel_t = cpool.tile([128, R], f32r)
        nc.sync.dma_start(sel_t[:], d_sel.ap().bitcast(f32r))
        id_t = cpool.tile([128, 128], f32r)
        nc.sync.dma_start(id_t[:], d_id.ap().bitcast(f32r))

        # ---- working tiles (exponent path f32, angular path bf16) ----
        tij2 = gpool.tile([128, W], f32)
        tik2 = gpool.tile([128, W], f32)
        tjk2 = gpool.tile([128, W], f32)
        s12 = gpool.tile([128, W], f32)
        num = gpool.tile([128, W], f32)
        den = gpool.tile([128, W], f32)
        rden = gpool.tile([128, W], f32)
        hh = gpool.tile([128, W], f32)
        s1a = gpool.tile([128, W], f32)
        c1 = gpool.tile([128, W], bf16)
        c2 = gpool.tile([128, W], bf16)
        c3 = gpool.tile([128, W], bf16)
        p12 = gpool.tile([128, W], bf16)
        p2 = gpool.tile([128, W], bf16)
        cm = gpool.tile([128, W], bf16)
        u1 = gpool.tile([128, W], bf16)
        v1 = gpool.tile([128, W], bf16)
        u2 = gpool.tile([128, W], bf16)
        v2 = gpool.tile([128, W], bf16)
        u4 = gpool.tile([128, W], bf16)
        v4 = gpool.tile([128, W], bf16)
        u8 = gpool.tile([128, W], bf16)
        v8 = gpool.tile([128, W], bf16)
        pall = gpool.tile([128, F * W], bf16)
        pall_s = pall[:].rearrange("p (f col) -> p col f", f=F)

        src_t = gpool.tile([128, NV], f32r)
        src3 = src_t[:].rearrange("p (pair k) -> p pair k", k=KRP)
        nc.gpsimd.memset(src_t[:].bitcast(f32), 0.0)
        ones_t = gpool.tile([128, PP], f32)
        nc.vector.memset(ones_t[:], 1.0)
        nc.vector.tensor_copy(src3[:, :, 2 * nch:2 * nch + 1],
                              ones_t[:].rearrange("p (pair k) -> p pair k", k=1))
        ts_t = gpool.tile([128, NV], f32r)
        s1_3 = src3[:, :, 0:nch]
        s2_3 = src3[:, :, nch:2 * nch]

        outs_t = gpool.tile([R, PP * F], f32)

        # ---- scalar transcendentals for ALL chunks first (2 table loads) ----
        for c in range(NCHK):
            sl = csl(c)
            nc.scalar.activation(c1[:, sl], rij_t[:, sl], Act.Sin,
                                 scale=PI / 10.0, bias=bias_hpi[:])
            nc.scalar.activation(c2[:, sl], rik_t[:, sl], Act.Sin,
                                 scale=PI / 10.0, bias=bias_hpi[:])
            nc.scalar.activation(c3[:, sl], rjk_t[:, sl], Act.Sin,
                                 scale=PI / 10.0, bias=bias_hpi[:])

        def src_chunk(c):
            # short path feeding the exponent matmuls: squares + sums only
            sl = csl(c)
            pr = slice(PCHK[c][0], PCHK[c][1])
            nc.gpsimd.tensor_tensor(tij2[:, sl], rij_t[:, sl], rij_t[:, sl], Alu.mult)
            nc.gpsimd.tensor_tensor(tik2[:, sl], rik_t[:, sl], rik_t[:, sl], Alu.mult)
            nc.vector.tensor_tensor(tjk2[:, sl], rjk_t[:, sl], rjk_t[:, sl], Alu.mult)
            nc.vector.tensor_tensor(s12[:, sl], tij2[:, sl], tik2[:, sl], Alu.add)
            nc.gpsimd.tensor_tensor(s1a[:, sl], rij_t[:, sl], rik_t[:, sl], Alu.add)
            nc.gpsimd.tensor_tensor(
                s1_3[:, pr, :],
                s1a[:, sl].rearrange("p (pair j) -> p pair j", j=nch),
                rjk_t[:, sl].rearrange("p (pair j) -> p pair j", j=nch), Alu.add)
            nc.vector.tensor_tensor(
                s2_3[:, pr, :],
                s12[:, sl].rearrange("p (pair j) -> p pair j", j=nch),
                tjk2[:, sl].rearrange("p (pair j) -> p pair j", j=nch), Alu.add)

        def ang_chunk(c):
            # angular/cutoff chain: bf16 ops mostly on Vector (2x modes),
            # f32 ops mostly on GpSimd (dtype-blind)
            sl = csl(c)
            nc.gpsimd.tensor_tensor(den[:, sl], rij_t[:, sl], rik_t[:, sl], Alu.mult)
            nc.vector.reciprocal_approx_fast(rden[:, sl], den[:, sl])
            nc.vector.scalar_tensor_tensor(num[:, sl], tjk2[:, sl], -1.0, s12[:, sl],
                                           Alu.mult, Alu.add)
            nc.vector.scalar_tensor_tensor(hh[:, sl], num[:, sl], -0.25, rden[:, sl],
                                           Alu.mult, Alu.mult)
            nc.vector.tensor_scalar(u1[:, sl], hh[:, sl], 0.5, None, Alu.add)
            nc.vector.tensor_scalar(v1[:, sl], hh[:, sl], -1.0, 0.5,
                                    Alu.mult, Alu.add)
            nc.gpsimd.tensor_tensor(p12[:, sl], c1[:, sl], c2[:, sl], Alu.mult)
            nc.gpsimd.tensor_tensor(p2[:, sl], p12[:, sl], c3[:, sl], Alu.mult)
            nc.vector.scalar_tensor_tensor(cm[:, sl], p2[:, sl], 2.0, p2[:, sl],
                                           Alu.mult, Alu.mult)
            nc.gpsimd.tensor_tensor(u2[:, sl], u1[:, sl], u1[:, sl], Alu.mult)
            nc.vector.tensor_tensor(v2[:, sl], v1[:, sl], v1[:, sl], Alu.mult)
            nc.gpsimd.tensor_tensor(u4[:, sl], u2[:, sl], u2[:, sl], Alu.mult)
            nc.vector.tensor_tensor(v4[:, sl], v2[:, sl], v2[:, sl], Alu.mult)
            nc.gpsimd.tensor_tensor(u8[:, sl], u4[:, sl], u4[:, sl], Alu.mult)
            nc.vector.tensor_tensor(v8[:, sl], v4[:, sl], v4[:, sl], Alu.mult)
            lo, hi = PCHK[c][0] * nch, PCHK[c][1] * nch
            for fi, pw in enumerate((u1, u2, u4, u8, v1, v2, v4, v8)):
                eng = nc.vector if fi % 2 == 0 else nc.gpsimd
                eng.tensor_tensor(pall[:, fi * W + lo:fi * W + hi],
                                  pw[:, sl], cm[:, sl], Alu.mult)

        _pc_tiles = {}

        def get_pc(t):
            if t not in _pc_tiles:
                _pc_tiles[t] = pcpool.tile([128, 128], f32, name=f"pc{t}", tag="pc")
                nc.vector.memset(_pc_tiles[t][:], 0.0)
            return _pc_tiles[t]

        # --- phase 1: per chunk: src tiles + transposes (short dep path) ---
        rads = [None] * NB
        for c in range(NCHK):
            src_chunk(c)
            for blk in range(PCHK[c][0] // 4, PCHK[c][1] // 4):
                pst = p2pool.tile([128, 128], f32r, name=f"pst{blk}", tag="ps2")
                nc.tensor.transpose(pst[:], src_t[:, blk * 128:(blk + 1) * 128],
                                    id_t[:])
                nc.vector.tensor_copy(ts_t[:, blk * 128:(blk + 1) * 128], pst[:])

        # --- phase 2: exponent matmuls + EXPs ---
        for blk in range(NB):
            rad = rpool.tile([128, 4 * WTOT], bf16, name=f"rad{blk}", tag="rad")
            rads[blk] = rad
            for e in range(4):
                psu = pupool.tile([128, 512], f32, name=f"psu{blk}_{e}",
                                  tag="psu")
                p0 = 32 * e
                nc.tensor.matmul(psu[:, 0:WTOT],
                                 ts_t[p0:p0 + KR, blk * 128:(blk + 1) * 128],
                                 bd_t[p0:p0 + KR, :],
                                 start=True, stop=True,
                                 tile_position=(p0, 0))
                nc.scalar.activation(rad[:, e * WTOT:(e + 1) * WTOT],
                                     psu[:, 0:WTOT], Act.Exp, bias=bias0[:])

        # --- phase 3: angular chain + accumulation matmuls per chunk ---
        for c in range(NCHK):
            ang_chunk(c)
            b0, b1 = PCHK[c][0] // 4, PCHK[c][1] // 4
            for blk in range(b0, b1):
                rad = rads[blk]
                for e in range(4):
                    pair = blk * 4 + e
                    t = pair // 16
                    q = pair % 16
                    pc = get_pc(t)
                    for j in range(nch):
                        g = gof[j]
                        nc.tensor.matmul(
                            pc[32 * g:32 * g + W0, q * F:(q + 1) * F],
                            rad[:, e * WTOT + j * W0:e * WTOT + (j + 1) * W0],
                            pall_s[:, pair * nch + j, :],
                            start=(j == gfirst[g]), stop=(j == glast[g]),
                            tile_position=(0, 32 * g),
                        )
                if blk % 4 == 3:
                    t = blk // 4
                    pc = _pc_tiles[t]
                    sb = gpool.tile([128, 128], f32r, name=f"sb{t}", tag="sb")
                    nc.vector.tensor_copy(sb[:], pc[:])
                    ps2 = p2pool.tile([R, 128], f32, name=f"ps2_{t}", tag="ps2")
                    nc.tensor.matmul(ps2[:], sel_t[:], sb[:], start=True, stop=True)
                    nc.vector.tensor_copy(outs_t[:, t * 128:(t + 1) * 128], ps2[:])
                    nc.sync.dma_start(d_out.ap()[:, t * 128:(t + 1) * 128],
                                      outs_t[:, t * 128:(t + 1) * 128])

    nc.compile()
    return nc


def _windows(rbar_sorted, nch):
    """Per-chunk offset windows -> 4 groups sharing an exact uniform window."""
    import itertools
    pos = (rbar_sorted - 0.5) / DELTA
    los, his = [], []
    for j in range(nch):
        blk = pos[:, j * 128:(j + 1) * 128]
        los.append(max(0, int(np.floor(blk.min() - RADI))))
        his.append(min(31, int(np.ceil(blk.max() + RADI))))
    best = None
    for splits in itertools.combinations(range(1, nch), 3):
        bnds = [0] + list(splits) + [nch]
        ws, ok, tot = [], True, 0
        for a, b in zip(bnds[:-1], bnds[1:]):
            lo, hi = min(los[a:b]), max(his[a:b])
            if hi - lo + 1 > 32:
                ok = False
                break
            ws.append((lo, hi))
            tot += (hi - lo + 1) * (b - a)
        if ok and (best is None or tot < best[0]):
            best = (tot, bnds, ws)
    if best is None:                      # fallback: full grid
        bnds, ws = [0, nch // 4, nch // 2, 3 * nch // 4, nch], [(0, 31)] * 4
    else:
        bnds, ws = best[1], best[2]
    W0 = max(hi - lo + 1 for lo, hi in ws)
    W0 += W0 % 2            # f32r matmul needs an even column count
    glo = [max(0, min(lo, 32 - W0)) for lo, hi in ws]
    gof = []
    for g in range(4):
        gof += [g] * (bnds[g + 1] - bnds[g])
    gfirst = [bnds[g] for g in range(4)]
    glast = [bnds[g + 1] - 1 for g in range(4)]
    return W0, tuple(glo), tuple(gof), tuple(gfirst), tuple(glast)


def _prep(r_ij, r_ik, r_jk, offsets, triple_masks):
    """Host-side shard + compact + sort-by-rbar + pad + transpose."""
    B, A, N = r_ij.shape
    P = B * A
    rij = np.ascontiguousarray(r_ij, dtype=np.float32).reshape(P, N)
    rik = np.ascontiguousarray(r_ik, dtype=np.float32).reshape(P, N)
    rjk = np.ascontiguousarray(r_jk, dtype=np.float32).reshape(P, N)
    m = (np.asarray(triple_masks).reshape(P, N) != 0)

    counts = m.sum(axis=1)
    npad = max(128, int(-(-max(1, counts.max()) // 128) * 128))
    nch = npad // 128

    cij = np.full((P, npad), 5.0, dtype=np.float32)
    cik = np.full((P, npad), 5.0, dtype=np.float32)
    cjk = np.full((P, npad), 5.0, dtype=np.float32)
    for p in range(P):
        idx = np.nonzero(m[p])[0]
        k = idx.size
        cij[p, :k] = rij[p, idx]
        cik[p, :k] = rik[p, idx]
        cjk[p, :k] = rjk[p, idx]

    order = np.argsort(cij + cik + cjk, axis=1, kind="stable")
    cij = np.take_along_axis(cij, order, 1)
    cik = np.take_along_axis(cik, order, 1)
    cjk = np.take_along_axis(cjk, order, 1)
    rbar = (cij + cik + cjk) / 3.0

    W0, glo, gof, gfirst, glast = _windows(rbar, nch)
    cfg = (nch, W0, glo, gof, gfirst, glast)

    o = np.asarray(offsets, dtype=np.float64)
    WTOT = nch * W0
    bd = np.zeros((128, WTOT), dtype=np.float32)
    for g4 in range(4):
        for j in range(nch):
            osl = o[glo[gof[j]]:glo[gof[j]] + W0]
            bd[32 * g4 + j, j * W0:(j + 1) * W0] = 2.0 * GAMMA * osl
            bd[32 * g4 + nch + j, j * W0:(j + 1) * W0] = -GAMMA
            bd[32 * g4 + 2 * nch, j * W0:(j + 1) * W0] = -3.0 * GAMMA * osl * osl
    sel = np.zeros((128, R), dtype=np.float32)
    for g4 in range(4):
        for rl in range(W0):
            sel[32 * g4 + rl, glo[g4] + rl] = 1.0

    def core_xpose(x, lo, hi):
        # chunk-major: X[p, pair*nch+j] = x[pair, j*128+p]
        return np.ascontiguousarray(
            x[lo:hi].reshape(PP, nch, 128).transpose(2, 0, 1).reshape(128, PP * nch))

    in_maps = []
    for c in range(N_CORES):
        lo, hi = c * PP, (c + 1) * PP
        in_maps.append({
            "rij": core_xpose(cij, lo, hi),
            "rik": core_xpose(cik, lo, hi),
            "rjk": core_xpose(cjk, lo, hi),
            "bdiag": bd, "sel": sel, "ident": np.eye(128, dtype=np.float32),
        })
    return in_maps, cfg


def _ensure_ntff_hook():
    """Register the axon NTFF profile hook if the image's antenv lacks it."""
    import types
    try:
        from antenv.axon_hooks import get_axon_ntff_profile_hook  # noqa: F401
        return
    except ImportError:
        pass
    try:
        sys.path.insert(0, "/root/.axon_site")
        from trn_agent_boot.trn_boot import _ntff_profile_via_ctypes
        hook = _ntff_profile_via_ctypes("/opt/axon/libaxon_pjrt.so")
        import antenv
        mod = types.ModuleType("antenv.axon_hooks")
        _holder = {"h": hook}
        mod.set_axon_ntff_profile_hook = lambda h: _holder.update(h=h)
        mod.get_axon_ntff_profile_hook = lambda: _holder["h"]
        sys.modules["antenv.axon_hooks"] = mod
        antenv.axon_hooks = mod
    except Exception:
        pass


def kernel(r_ij, r_ik, r_jk, offsets, triple_masks):
    global LAST_EXEC_NS
    from concourse.bass_utils import run_bass_kernel_spmd
    _ensure_ntff_hook()

    B, A, N = r_ij.shape
    in_maps, cfg = _prep(r_ij, r_ik, r_jk, offsets, triple_masks)
    if cfg not in _CACHE:
        _CACHE[cfg] = _build(cfg)
    nc = _CACHE[cfg]

    trace = os.environ.get("KERNEL_TRACE", "0") == "1"
    res = run_bass_kernel_spmd(nc, in_maps, core_ids=list(range(N_CORES)),
                               trace=trace)
    LAST_EXEC_NS = res.exec_time_ns
    outs = []
    for r in res.results:
        # [32, 512]: cols pair*8+f
        a = r["out"].reshape(R, PP, F).transpose(1, 0, 2).reshape(PP, R * F)
        outs.append(a)
    out = np.concatenate(outs, axis=0)
    return out.reshape(B, A, R * F)


# revision 34
# speedup vs baseline: 1.0782x; 1.0782x over previous
"""AngularDistribution Trainium2 kernel (8 NeuronCores, SPMD over (batch,atom) pairs).

Math (per pair p, triple n, offset r, filter f):
  rad[n,r]  = exp(-g*(S2[n] - 2*o_r*S1[n] + 3*o_r^2))   S1=rij+rik+rjk, S2=sum sq
  ang[n,f]  = 2*u^z (f<4, u=(1-ct)/2) or 2*v^z (f>=4), z in {1,2,4,8}
  cm[n]     = 2*(cos(pi*rij/10)*cos(pi*rik/10)*cos(pi*rjk/10))^2
  out[p,r*8+f] = sum_n rad[n,r] * pw[n,f] * cm[n]    (mask via host compaction)

v4 structure per core (64 pairs, nch*128 triples padded, sorted by rbar):
  - host compacts, SORTS each pair's triples by rbar=(S1/3), pre-transposes to
    [128, 64*nch] (chunk-major: chunk j = rbar quantile block), contiguous DMA
  - rad[n, r] is ~zero outside |o_r - rbar| < 0.65, so each chunk only needs a
    W0-wide offset window; chunks are grouped into 4 groups sharing an exact
    window [lo_g, lo_g+W0) -> exponent matmul streams nch*W0 cols (not nch*32)
  - tiny matmuls accumulate window-LOCAL into 4 aligned 32-row PSUM bands;
    a constant 0/1 matrix un-shifts bands to global r via one matmul per 16
    pairs
  - S1/S2 source tile transposed on the PE in f32r (half the f32 cost)
  - power/cutoff chain in bf16 (2x DVE modes); exponent path stays f32
"""

import os
import sys

sys.path.insert(0, "/opt/trn_rl_repo")

import numpy as np
from contextlib import ExitStack

GAMMA = 4.0
N_CORES = 8
PP = 64          # pairs per core (512 total / 8)
R = 32
F = 8
DELTA = 4.5 / 31  # offset grid spacing
RADI = 4.5        # window radius in grid units (= 0.65 distance)

_CACHE = {}
LAST_EXEC_NS = None


def _build(cfg):
    nch, W0, lo_of_chunk, gof, gfirst, glast = cfg
    import concourse.bass as bass
    import concourse.tile as tile
    from concourse import bacc, mybir

    f32 = mybir.dt.float32
    f32r = mybir.dt.float32r
    bf16 = mybir.dt.bfloat16
    Alu = mybir.AluOpType
    Act = mybir.ActivationFunctionType
    W = PP * nch          # global tile free size
    KR = 2 * nch + 1      # lhsT rows per pair (S1 rows, S2 rows, ones row)
    KRP = 32              # padded row stride: 4 pairs per 128-row block
    NV = PP * KRP
    NB = NV // 128        # 128-col blocks in src/ts (== PP//4)
    WTOT = nch * W0       # exponent grid cols per pair (windowed)
    SPC = 512             # psu column spacing per pair (bank-aligned)
    PI = float(np.pi)
    assert WTOT <= SPC

    nc = bacc.Bacc("TRN2", target_bir_lowering=False, debug=False,
                   num_devices=N_CORES)

    d_rij = nc.dram_tensor("rij", [128, W], f32, kind="ExternalInput")
    d_rik = nc.dram_tensor("rik", [128, W], f32, kind="ExternalInput")
    d_rjk = nc.dram_tensor("rjk", [128, W], f32, kind="ExternalInput")
    d_bd = nc.dram_tensor("bdiag", [128, WTOT], f32, kind="ExternalInput")
    d_sel = nc.dram_tensor("sel", [128, R], f32, kind="ExternalInput")
    d_id = nc.dram_tensor("ident", [128, 128], f32, kind="ExternalInput")
    d_out = nc.dram_tensor("out", [R, PP * F], f32, kind="ExternalOutput")

    with tile.TileContext(nc) as tc, ExitStack() as ctx:
        cpool = ctx.enter_context(tc.tile_pool(name="consts", bufs=1))
        gpool = ctx.enter_context(tc.tile_pool(name="glob", bufs=1))
        rpool = ctx.enter_context(tc.tile_pool(name="rad", bufs=16))
        pupool = ctx.enter_context(tc.tile_pool(name="psu", bufs=2, space="PSUM"))
        pcpool = ctx.enter_context(tc.tile_pool(name="pc", bufs=2, space="PSUM"))
        p2pool = ctx.enter_context(tc.tile_pool(name="ps2", bufs=2, space="PSUM"))

        bias0 = cpool.tile([128, 1], f32)
        nc.vector.memset(bias0[:], 0.0)
        bias_hpi = cpool.tile([128, 1], f32)
        nc.vector.memset(bias_hpi[:], PI / 2.0)

        # ---- input tiles, contiguous DMA in pair chunks (small first) ----
        rij_t = gpool.tile([128, W], f32)
        rik_t = gpool.tile([128, W], f32)
        rjk_t = gpool.tile([128, W], f32)
        PCHK = [(0, 8), (8, 32), (32, 64)]
        NCHK = len(PCHK)

        def csl(c):
            return slice(PCHK[c][0] * nch, PCHK[c][1] * nch)

        for c in range(NCHK):
            sl = csl(c)
            for dst, src in ((rij_t, d_rij), (rik_t, d_rik), (rjk_t, d_rjk)):
                nc.sync.dma_start(dst[:, sl], src.ap()[:, sl])
        bd_t = cpool.tile([128, WTOT], f32r)
        nc.sync.dma_start(bd_t[:], d_bd.ap().bitcast(f32r))
        sel_t = cpool.tile([128, R], f32r)
        nc.sync.dma_start(sel_t[:], d_sel.ap().bitcast(f32r))
        id_t = cpool.tile([128, 128], f32r)
        nc.sync.dma_start(id_t[:], d_id.ap().bitcast(f32r))

        # ---- working tiles (exponent path f32, angular path bf16) ----
        tij2 = gpool.tile([128, W], f32)
        tik2 = gpool.tile([128, W], f32)
        tjk2 = gpool.tile([128, W], f32)
        s12 = gpool.tile([128, W], f32)
        num = gpool.tile([128, W], f32)
        den = gpool.tile([128, W], f32)
        rden = gpool.tile([128, W], f32)
        hh = gpool.tile([128, W], f32)
        s1a = gpool.tile([128, W], f32)
        c1 = gpool.tile([128, W], bf16)
        c2 = gpool.tile([128, W], bf16)
        c3 = gpool.tile([128, W], bf16)
        p12 = gpool.tile([128, W], bf16)
        p2 = gpool.tile([128, W], bf16)
        cm = gpool.tile([128, W], bf16)
        u1 = gpool.tile([128, W], bf16)
        v1 = gpool.tile([128, W], bf16)
        u2 = gpool.tile([128, W], bf16)
        v2 = gpool.tile([128, W], bf16)
        u4 = gpool.tile([128, W], bf16)
        v4 = gpool.tile([128, W], bf16)
        u8 = gpool.tile([128, W], bf16)
        v8 = gpool.tile([128, W], bf16)
        pall = gpool.tile([128, F * W], bf16)
        pall_s = pall[:].rearrange("p (f col) -> p col f", f=F)

        src_t = gpool.tile([128, NV], f32r)
        src3 = src_t[:].rearrange("p (pair k) -> p pair k", k=KRP)
        nc.gpsimd.memset(src_t[:].bitcast(f32), 0.0)
        ones_t = gpool.tile([128, PP], f32)
        nc.vector.memset(ones_t[:], 1.0)
        nc.vector.tensor_copy(src3[:, :, 2 * nch:2 * nch + 1],
                              ones_t[:].rearrange("p (pair k) -> p pair k", k=1))
        ts_t = gpool.tile([128, NV], f32r)
        s1_3 = src3[:, :, 0:nch]
        s2_3 = src3[:, :, nch:2 * nch]

        outs_t = gpool.tile([R, PP * F], f32)

        # ---- scalar transcendentals for ALL chunks first (2 table loads) ----
        for c in range(NCHK):
            sl = csl(c)
            nc.scalar.activation(c1[:, sl], rij_t[:, sl], Act.Sin,
                                 scale=PI / 10.0, bias=bias_hpi[:])
            nc.scalar.activation(c2[:, sl], rik_t[:, sl], Act.Sin,
                                 scale=PI / 10.0, bias=bias_hpi[:])
            nc.scalar.activation(c3[:, sl], rjk_t[:, sl], Act.Sin,
                                 scale=PI / 10.0, bias=bias_hpi[:])

        def src_chunk(c):
            # short path feeding the exponent matmuls: squares + sums only
            sl = csl(c)
            pr = slice(PCHK[c][0], PCHK[c][1])
            nc.gpsimd.tensor_tensor(tij2[:, sl], rij_t[:, sl], rij_t[:, sl], Alu.mult)
            nc.gpsimd.tensor_tensor(tik2[:, sl], rik_t[:, sl], rik_t[:, sl], Alu.mult)
            nc.vector.tensor_tensor(tjk2[:, sl], rjk_t[:, sl], rjk_t[:, sl], Alu.mult)
            nc.vector.tensor_tensor(s12[:, sl], tij2[:, sl], tik2[:, sl], Alu.add)
            nc.gpsimd.tensor_tensor(s1a[:, sl], rij_t[:, sl], rik_t[:, sl], Alu.add)
            nc.gpsimd.tensor_tensor(
                s1_3[:, pr, :],
                s1a[:, sl].rearrange("p (pair j) -> p pair j", j=nch),
                rjk_t[:, sl].rearrange("p (pair j) -> p pair j", j=nch), Alu.add)
            nc.vector.tensor_tensor(
                s2_3[:, pr, :],
                s12[:, sl].rearrange("p (pair j) -> p pair j", j=nch),
                tjk2[:, sl].rearrange("p (pair j) -> p pair j", j=nch), Alu.add)

        def ang_chunk(c):
            # angular/cutoff chain: bf16 ops mostly on Vector (2x modes),
            # f32 ops mostly on GpSimd (dtype-blind)
            sl = csl(c)
            nc.gpsimd.tensor_tensor(den[:, sl], rij_t[:, sl], rik_t[:, sl], Alu.mult)
            nc.vector.reciprocal_approx_fast(rden[:, sl], den[:, sl])
            nc.vector.scalar_tensor_tensor(num[:, sl], tjk2[:, sl], -1.0, s12[:, sl],
                                           Alu.mult, Alu.add)
            nc.vector.scalar_tensor_tensor(hh[:, sl], num[:, sl], -0.25, rden[:, sl],
                                           Alu.mult, Alu.mult)
            nc.vector.tensor_scalar(u1[:, sl], hh[:, sl], 0.5, None, Alu.add)
            nc.vector.tensor_scalar(v1[:, sl], hh[:, sl], -1.0, 0.5,
                                    Alu.mult, Alu.add)
            nc.gpsimd.tensor_tensor(p12[:, sl], c1[:, sl], c2[:, sl], Alu.mult)
            nc.gpsimd.tensor_tensor(p2[:, sl], p12[:, sl], c3[:, sl], Alu.mult)
            nc.vector.scalar_tensor_tensor(cm[:, sl], p2[:, sl], 2.0, p2[:, sl],
                                           Alu.mult, Alu.mult)
            nc.gpsimd.tensor_tensor(u2[:, sl], u1[:, sl], u1[:, sl], Alu.mult)
            nc.vector.tensor_tensor(v2[:, sl], v1[:, sl], v1[:, sl], Alu.mult)
            nc.gpsimd.tensor_tensor(u4[:, sl], u2[:, sl], u2[:, sl], Alu.mult)
            nc.vector.tensor_tensor(v4[:, sl], v2[:, sl], v2[:, sl], Alu.mult)
            nc.gpsimd.tensor_tensor(u8[:, sl], u4[:, sl], u4[:, sl], Alu.mult)
            nc.vector.tensor_tensor(v8[:, sl], v4[:, sl], v4[:, sl], Alu.mult)
            lo, hi = PCHK[c][0] * nch, PCHK[c][1] * nch
            for fi, pw in enumerate((u1, u2, u4, u8, v1, v2, v4, v8)):
                eng = nc.vector if fi % 2 == 0 else nc.gpsimd
                eng.tensor_tensor(pall[:, fi * W + lo:fi * W + hi],
                                  pw[:, sl], cm[:, sl], Alu.mult)

        _pc_tiles = {}

        def get_pc(t):
            if t not in _pc_tiles:
                _pc_tiles[t] = pcpool.tile([128, 128], f32, name=f"pc{t}", tag="pc")
                nc.vector.memset(_pc_tiles[t][:], 0.0)
            return _pc_tiles[t]

        # --- phase 1: per chunk: src tiles + transposes (short dep path) ---
        rads = [None] * NB
        for c in range(NCHK):
            src_chunk(c)
            for blk in range(PCHK[c][0] // 4, PCHK[c][1] // 4):
                pst = p2pool.tile([128, 128], f32r, name=f"pst{blk}", tag="ps2")
                nc.tensor.transpose(pst[:], src_t[:, blk * 128:(blk + 1) * 128],
                                    id_t[:])
                nc.vector.tensor_copy(ts_t[:, blk * 128:(blk + 1) * 128], pst[:])

        # --- phase 2: exponent matmuls + EXPs ---
        for blk in range(NB):
            rad = rpool.tile([128, 4 * WTOT], bf16, name=f"rad{blk}", tag="rad")
            rads[blk] = rad
            for half in range(2):
                psu = pupool.tile([128, 1024], f32, name=f"psu{blk}_{half}",
                                  tag="psu")
                for e in range(2):
                    p0 = 32 * (half * 2 + e)
                    nc.tensor.matmul(psu[:, e * SPC:e * SPC + WTOT],
                                     ts_t[p0:p0 + KR, blk * 128:(blk + 1) * 128],
                                     bd_t[p0:p0 + KR, :],
                                     start=True, stop=True,
                                     tile_position=(p0, 0))
                nc.scalar.activation(
                    rad[:].rearrange("p (e g) -> p e g", e=4)
                        [:, 2 * half:2 * half + 2, :],
                    psu[:].rearrange("p (e g) -> p e g", e=2)[:, :, 0:WTOT],
                    Act.Exp, bias=bias0[:])

        # --- phase 3: angular chain + accumulation matmuls per chunk ---
        for c in range(NCHK):
            ang_chunk(c)
            b0, b1 = PCHK[c][0] // 4, PCHK[c][1] // 4
            for blk in range(b0, b1):
                rad = rads[blk]
                for e in range(4):
                    pair = blk * 4 + e
                    t = pair // 16
                    q = pair % 16
                    pc = get_pc(t)
                    for j in range(nch):
                        g = gof[j]
                        nc.tensor.matmul(
                            pc[32 * g:32 * g + W0, q * F:(q + 1) * F],
                            rad[:, e * WTOT + j * W0:e * WTOT + (j + 1) * W0],
                            pall_s[:, pair * nch + j, :],
                            start=(j == gfirst[g]), stop=(j == glast[g]),
                            tile_position=(0, 32 * g),
                        )
                if blk % 4 == 3:
                    t = blk // 4
                    pc = _pc_tiles[t]
                    sb = gpool.tile([128, 128], f32r, name=f"sb{t}", tag="sb")
                    nc.vector.tensor_copy(sb[:], pc[:])
                    ps2 = p2pool.tile([R, 128], f32, name=f"ps2_{t}", tag="ps2")
                    nc.tensor.matmul(ps2[:], sel_t[:], sb[:], start=True, stop=True)
                    nc.vector.tensor_copy(outs_t[:, t * 128:(t + 1) * 128], ps2[:])
                    nc.sync.dma_start(d_out.ap()[:, t * 128:(t + 1) * 128],
                                      outs_t[:, t * 128:(t + 1) * 128])

    nc.compile()
    return nc


def _windows(rbar_sorted, nch):
    """Per-chunk offset windows -> 4 groups sharing an exact uniform window."""
    import itertools
    pos = (rbar_sorted - 0.5) / DELTA
    los, his = [], []
    for j in range(nch):
        blk = pos[:, j * 128:(j + 1) * 128]
        los.append(max(0, int(np.floor(blk.min() - RADI))))
        his.append(min(31, int(np.ceil(blk.max() + RADI))))
    best = None
    for splits in itertools.combinations(range(1, nch), 3):
        bnds = [0] + list(splits) + [nch]
        ws, ok, tot = [], True, 0
        for a, b in zip(bnds[:-1], bnds[1:]):
            lo, hi = min(los[a:b]), max(his[a:b])
            if hi - lo + 1 > 32:
                ok = False
                break
            ws.append((lo, hi))
            tot += (hi - lo + 1) * (b - a)
        if ok and (best is None or tot < best[0]):
            best = (tot, bnds, ws)
    if best is None:                      # fallback: full grid
        bnds, ws = [0, nch // 4, nch // 2, 3 * nch // 4, nch], [(0, 31)] * 4
    else:
        bnds, ws = best[1], best[2]
    W0 = max(hi - lo + 1 for lo, hi in ws)
    W0 += W0 % 2            # f32r matmul needs an even column count
    glo = [max(0, min(lo, 32 - W0)) for lo, hi in ws]
    gof = []
    for g in range(4):
        gof += [g] * (bnds[g + 1] - bnds[g])
    gfirst = [bnds[g] for g in range(4)]
    glast = [bnds[g + 1] - 1 for g in range(4)]
    return W0, tuple(glo), tuple(gof), tuple(gfirst), tuple(glast)


def _prep(r_ij, r_ik, r_jk, offsets, triple_masks):
    """Host-side shard + compact + sort-by-rbar + pad + transpose."""
    B, A, N = r_ij.shape
    P = B * A
    rij = np.ascontiguousarray(r_ij, dtype=np.float32).reshape(P, N)
    rik = np.ascontiguousarray(r_ik, dtype=np.float32).reshape(P, N)
    rjk = np.ascontiguousarray(r_jk, dtype=np.float32).reshape(P, N)
    m = (np.asarray(triple_masks).reshape(P, N) != 0)

    counts = m.sum(axis=1)
    npad = max(128, int(-(-max(1, counts.max()) // 128) * 128))
    nch = npad // 128

    cij = np.full((P, npad), 5.0, dtype=np.float32)
    cik = np.full((P, npad), 5.0, dtype=np.float32)
    cjk = np.full((P, npad), 5.0, dtype=np.float32)
    for p in range(P):
        idx = np.nonzero(m[p])[0]
        k = idx.size
        cij[p, :k] = rij[p, idx]
        cik[p, :k] = rik[p, idx]
        cjk[p, :k] = rjk[p, idx]

    order = np.argsort(cij + cik + cjk, axis=1, kind="stable")
    cij = np.take_along_axis(cij, order, 1)
    cik = np.take_along_axis(cik, order, 1)
    cjk = np.take_along_axis(cjk, order, 1)
    rbar = (cij + cik + cjk) / 3.0

    W0, glo, gof, gfirst, glast = _windows(rbar, nch)
    cfg = (nch, W0, glo, gof, gfirst, glast)

    o = np.asarray(offsets, dtype=np.float64)
    WTOT = nch * W0
    bd = np.zeros((128, WTOT), dtype=np.float32)
    for g4 in range(4):
        for j in range(nch):
            osl = o[glo[gof[j]]:glo[gof[j]] + W0]
            bd[32 * g4 + j, j * W0:(j + 1) * W0] = 2.0 * GAMMA * osl
            bd[32 * g4 + nch + j, j * W0:(j + 1) * W0] = -GAMMA
            bd[32 * g4 + 2 * nch, j * W0:(j + 1) * W0] = -3.0 * GAMMA * osl * osl
    sel = np.zeros((128, R), dtype=np.float32)
    for g4 in range(4):
        for rl in range(W0):
            sel[32 * g4 + rl, glo[g4] + rl] = 1.0

    def core_xpose(x, lo, hi):
        # chunk-major: X[p, pair*nch+j] = x[pair, j*128+p]
        return np.ascontiguousarray(
            x[lo:hi].reshape(PP, nch, 128).transpose(2, 0, 1).reshape(128, PP * nch))

    in_maps = []
    for c in range(N_CORES):
        lo, hi = c * PP, (c + 1) * PP
        in_maps.append({
            "rij": core_xpose(cij, lo, hi),
            "rik": core_xpose(cik, lo, hi),
            "rjk": core_xpose(cjk, lo, hi),
            "bdiag": bd, "sel": sel, "ident": np.eye(128, dtype=np.float32),
        })
    return in_maps, cfg


def _ensure_ntff_hook():
    """Register the axon NTFF profile hook if the image's antenv lacks it."""
    import types
    try:
        from antenv.axon_hooks import get_axon_ntff_profile_hook  # noqa: F401
        return
    except ImportError:
        pass
    try:
        sys.path.insert(0, "/root/.axon_site")
        from trn_agent_boot.trn_boot import _ntff_profile_via_ctypes
        hook = _ntff_profile_via_ctypes("/opt/axon/libaxon_pjrt.so")
        import antenv
        mod = types.ModuleType("antenv.axon_hooks")
        _holder = {"h": hook}
        mod.set_axon_ntff_profile_hook = lambda h: _holder.update(h=h)
        mod.get_axon_ntff_profile_hook = lambda: _holder["h"]
        sys.modules["antenv.axon_hooks"] = mod
        antenv.axon_hooks = mod
    except Exception:
        pass


def kernel(r_ij, r_ik, r_jk, offsets, triple_masks):
    global LAST_EXEC_NS
    from concourse.bass_utils import run_bass_kernel_spmd
    _ensure_ntff_hook()

    B, A, N = r_ij.shape
    in_maps, cfg = _prep(r_ij, r_ik, r_jk, offsets, triple_masks)
    if cfg not in _CACHE:
        _CACHE[cfg] = _build(cfg)
    nc = _CACHE[cfg]

    trace = os.environ.get("KERNEL_TRACE", "0") == "1"
    res = run_bass_kernel_spmd(nc, in_maps, core_ids=list(range(N_CORES)),
                               trace=trace)
    LAST_EXEC_NS = res.exec_time_ns
    outs = []
    for r in res.results:
        # [32, 512]: cols pair*8+f
        a = r["out"].reshape(R, PP, F).transpose(1, 0, 2).reshape(PP, R * F)
        outs.append(a)
    out = np.concatenate(outs, axis=0)
    return out.reshape(B, A, R * F)


# revision 35
# speedup vs baseline: 1.1611x; 1.0769x over previous
"""AngularDistribution Trainium2 kernel (8 NeuronCores, SPMD over (batch,atom) pairs).

Math (per pair p, triple n, offset r, filter f):
  rad[n,r]  = exp(-g*(S2[n] - 2*o_r*S1[n] + 3*o_r^2))   S1=rij+rik+rjk, S2=sum sq
  ang[n,f]  = 2*u^z (f<4, u=(1-ct)/2) or 2*v^z (f>=4), z in {1,2,4,8}
  cm[n]     = 2*(cos(pi*rij/10)*cos(pi*rik/10)*cos(pi*rjk/10))^2
  out[p,r*8+f] = sum_n rad[n,r] * pw[n,f] * cm[n]    (mask via host compaction)

v4 structure per core (64 pairs, nch*128 triples padded, sorted by rbar):
  - host compacts, SORTS each pair's triples by rbar=(S1/3), pre-transposes to
    [128, 64*nch] (chunk-major: chunk j = rbar quantile block), contiguous DMA
  - rad[n, r] is ~zero outside |o_r - rbar| < 0.65, so each chunk only needs a
    W0-wide offset window; chunks are grouped into 4 groups sharing an exact
    window [lo_g, lo_g+W0) -> exponent matmul streams nch*W0 cols (not nch*32)
  - tiny matmuls accumulate window-LOCAL into 4 aligned 32-row PSUM bands;
    a constant 0/1 matrix un-shifts bands to global r via one matmul per 16
    pairs
  - S1/S2 source tile transposed on the PE in f32r (half the f32 cost)
  - power/cutoff chain in bf16 (2x DVE modes); exponent path stays f32
"""

import os
import sys

sys.path.insert(0, "/opt/trn_rl_repo")

import numpy as np
from contextlib import ExitStack

GAMMA = 4.0
N_CORES = 8
PP = 64          # pairs per core (512 total / 8)
R = 32
F = 8
DELTA = 4.5 / 31  # offset grid spacing
RADI = 4.0        # window radius in grid units (= 0.58 distance)

_CACHE = {}
LAST_EXEC_NS = None


def _build(cfg):
    nch, W0, lo_of_chunk, gof, gfirst, glast = cfg
    import concourse.bass as bass
    import concourse.tile as tile
    from concourse import bacc, mybir

    f32 = mybir.dt.float32
    f32r = mybir.dt.float32r
    bf16 = mybir.dt.bfloat16
    Alu = mybir.AluOpType
    Act = mybir.ActivationFunctionType
    W = PP * nch          # global tile free size
    KR = 2 * nch + 1      # lhsT rows per pair (S1 rows, S2 rows, ones row)
    KRP = 32              # padded row stride: 4 pairs per 128-row block
    NV = PP * KRP
    NB = NV // 128        # 128-col blocks in src/ts (== PP//4)
    WTOT = nch * W0       # exponent grid cols per pair (windowed)
    SPC = 512             # psu column spacing per pair (bank-aligned)
    PI = float(np.pi)
    assert WTOT <= SPC

    nc = bacc.Bacc("TRN2", target_bir_lowering=False, debug=False,
                   num_devices=N_CORES)

    d_rij = nc.dram_tensor("rij", [128, W], f32, kind="ExternalInput")
    d_rik = nc.dram_tensor("rik", [128, W], f32, kind="ExternalInput")
    d_rjk = nc.dram_tensor("rjk", [128, W], f32, kind="ExternalInput")
    d_bd = nc.dram_tensor("bdiag", [128, WTOT], f32, kind="ExternalInput")
    d_sel = nc.dram_tensor("sel", [128, R], f32, kind="ExternalInput")
    d_id = nc.dram_tensor("ident", [128, 128], f32, kind="ExternalInput")
    d_out = nc.dram_tensor("out", [R, PP * F], f32, kind="ExternalOutput")

    with tile.TileContext(nc) as tc, ExitStack() as ctx:
        cpool = ctx.enter_context(tc.tile_pool(name="consts", bufs=1))
        gpool = ctx.enter_context(tc.tile_pool(name="glob", bufs=1))
        rpool = ctx.enter_context(tc.tile_pool(name="rad", bufs=16))
        pupool = ctx.enter_context(tc.tile_pool(name="psu", bufs=2, space="PSUM"))
        pcpool = ctx.enter_context(tc.tile_pool(name="pc", bufs=2, space="PSUM"))
        p2pool = ctx.enter_context(tc.tile_pool(name="ps2", bufs=2, space="PSUM"))

        bias0 = cpool.tile([128, 1], f32)
        nc.vector.memset(bias0[:], 0.0)
        bias_hpi = cpool.tile([128, 1], f32)
        nc.vector.memset(bias_hpi[:], PI / 2.0)

        # ---- input tiles, contiguous DMA in pair chunks (small first) ----
        rij_t = gpool.tile([128, W], f32)
        rik_t = gpool.tile([128, W], f32)
        rjk_t = gpool.tile([128, W], f32)
        PCHK = [(0, 8), (8, 32), (32, 64)]
        NCHK = len(PCHK)

        def csl(c):
            return slice(PCHK[c][0] * nch, PCHK[c][1] * nch)

        for c in range(NCHK):
            sl = csl(c)
            for dst, src in ((rij_t, d_rij), (rik_t, d_rik), (rjk_t, d_rjk)):
                nc.sync.dma_start(dst[:, sl], src.ap()[:, sl])
        bd_t = cpool.tile([128, WTOT], f32r)
        nc.sync.dma_start(bd_t[:], d_bd.ap().bitcast(f32r))
        sel_t = cpool.tile([128, R], f32r)
        nc.sync.dma_start(sel_t[:], d_sel.ap().bitcast(f32r))
        id_t = cpool.tile([128, 128], f32r)
        nc.sync.dma_start(id_t[:], d_id.ap().bitcast(f32r))

        # ---- working tiles (exponent path f32, angular path bf16) ----
        tij2 = gpool.tile([128, W], f32)
        tik2 = gpool.tile([128, W], f32)
        tjk2 = gpool.tile([128, W], f32)
        s12 = gpool.tile([128, W], f32)
        num = gpool.tile([128, W], f32)
        den = gpool.tile([128, W], f32)
        rden = gpool.tile([128, W], f32)
        hh = gpool.tile([128, W], f32)
        s1a = gpool.tile([128, W], f32)
        c1 = gpool.tile([128, W], bf16)
        c2 = gpool.tile([128, W], bf16)
        c3 = gpool.tile([128, W], bf16)
        p12 = gpool.tile([128, W], bf16)
        p2 = gpool.tile([128, W], bf16)
        cm = gpool.tile([128, W], bf16)
        u1 = gpool.tile([128, W], bf16)
        v1 = gpool.tile([128, W], bf16)
        u2 = gpool.tile([128, W], bf16)
        v2 = gpool.tile([128, W], bf16)
        u4 = gpool.tile([128, W], bf16)
        v4 = gpool.tile([128, W], bf16)
        u8 = gpool.tile([128, W], bf16)
        v8 = gpool.tile([128, W], bf16)
        pall = gpool.tile([128, F * W], bf16)
        pall_s = pall[:].rearrange("p (f col) -> p col f", f=F)

        src_t = gpool.tile([128, NV], f32r)
        src3 = src_t[:].rearrange("p (pair k) -> p pair k", k=KRP)
        nc.gpsimd.memset(src_t[:].bitcast(f32), 0.0)
        ones_t = gpool.tile([128, PP], f32)
        nc.vector.memset(ones_t[:], 1.0)
        nc.vector.tensor_copy(src3[:, :, 2 * nch:2 * nch + 1],
                              ones_t[:].rearrange("p (pair k) -> p pair k", k=1))
        ts_t = gpool.tile([128, NV], f32r)
        s1_3 = src3[:, :, 0:nch]
        s2_3 = src3[:, :, nch:2 * nch]

        outs_t = gpool.tile([R, PP * F], f32)

        # ---- scalar transcendentals for ALL chunks first (2 table loads) ----
        for c in range(NCHK):
            sl = csl(c)
            nc.scalar.activation(c1[:, sl], rij_t[:, sl], Act.Sin,
                                 scale=PI / 10.0, bias=bias_hpi[:])
            nc.scalar.activation(c2[:, sl], rik_t[:, sl], Act.Sin,
                                 scale=PI / 10.0, bias=bias_hpi[:])
            nc.scalar.activation(c3[:, sl], rjk_t[:, sl], Act.Sin,
                                 scale=PI / 10.0, bias=bias_hpi[:])

        def src_chunk(c):
            # short path feeding the exponent matmuls: squares + sums only
            sl = csl(c)
            pr = slice(PCHK[c][0], PCHK[c][1])
            nc.gpsimd.tensor_tensor(tij2[:, sl], rij_t[:, sl], rij_t[:, sl], Alu.mult)
            nc.gpsimd.tensor_tensor(tik2[:, sl], rik_t[:, sl], rik_t[:, sl], Alu.mult)
            nc.vector.tensor_tensor(tjk2[:, sl], rjk_t[:, sl], rjk_t[:, sl], Alu.mult)
            nc.vector.tensor_tensor(s12[:, sl], tij2[:, sl], tik2[:, sl], Alu.add)
            nc.gpsimd.tensor_tensor(s1a[:, sl], rij_t[:, sl], rik_t[:, sl], Alu.add)
            nc.gpsimd.tensor_tensor(
                s1_3[:, pr, :],
                s1a[:, sl].rearrange("p (pair j) -> p pair j", j=nch),
                rjk_t[:, sl].rearrange("p (pair j) -> p pair j", j=nch), Alu.add)
            nc.vector.tensor_tensor(
                s2_3[:, pr, :],
                s12[:, sl].rearrange("p (pair j) -> p pair j", j=nch),
                tjk2[:, sl].rearrange("p (pair j) -> p pair j", j=nch), Alu.add)

        def ang_chunk(c):
            # angular/cutoff chain: bf16 ops mostly on Vector (2x modes),
            # f32 ops mostly on GpSimd (dtype-blind)
            sl = csl(c)
            nc.gpsimd.tensor_tensor(den[:, sl], rij_t[:, sl], rik_t[:, sl], Alu.mult)
            nc.vector.reciprocal_approx_fast(rden[:, sl], den[:, sl])
            nc.vector.scalar_tensor_tensor(num[:, sl], tjk2[:, sl], -1.0, s12[:, sl],
                                           Alu.mult, Alu.add)
            nc.vector.scalar_tensor_tensor(hh[:, sl], num[:, sl], -0.25, rden[:, sl],
                                           Alu.mult, Alu.mult)
            nc.vector.tensor_scalar(u1[:, sl], hh[:, sl], 0.5, None, Alu.add)
            nc.vector.tensor_scalar(v1[:, sl], hh[:, sl], -1.0, 0.5,
                                    Alu.mult, Alu.add)
            nc.gpsimd.tensor_tensor(p12[:, sl], c1[:, sl], c2[:, sl], Alu.mult)
            nc.gpsimd.tensor_tensor(p2[:, sl], p12[:, sl], c3[:, sl], Alu.mult)
            nc.vector.scalar_tensor_tensor(cm[:, sl], p2[:, sl], 2.0, p2[:, sl],
                                           Alu.mult, Alu.mult)
            nc.gpsimd.tensor_tensor(u2[:, sl], u1[:, sl], u1[:, sl], Alu.mult)
            nc.vector.tensor_tensor(v2[:, sl], v1[:, sl], v1[:, sl], Alu.mult)
            nc.gpsimd.tensor_tensor(u4[:, sl], u2[:, sl], u2[:, sl], Alu.mult)
            nc.vector.tensor_tensor(v4[:, sl], v2[:, sl], v2[:, sl], Alu.mult)
            nc.gpsimd.tensor_tensor(u8[:, sl], u4[:, sl], u4[:, sl], Alu.mult)
            nc.vector.tensor_tensor(v8[:, sl], v4[:, sl], v4[:, sl], Alu.mult)
            lo, hi = PCHK[c][0] * nch, PCHK[c][1] * nch
            for fi, pw in enumerate((u1, u2, u4, u8, v1, v2, v4, v8)):
                eng = nc.vector if fi % 2 == 0 else nc.gpsimd
                eng.tensor_tensor(pall[:, fi * W + lo:fi * W + hi],
                                  pw[:, sl], cm[:, sl], Alu.mult)

        _pc_tiles = {}

        def get_pc(t):
            if t not in _pc_tiles:
                _pc_tiles[t] = pcpool.tile([128, 128], f32, name=f"pc{t}", tag="pc")
                nc.vector.memset(_pc_tiles[t][:], 0.0)
            return _pc_tiles[t]

        # --- phase 1: per chunk: src tiles + transposes (short dep path) ---
        rads = [None] * NB
        for c in range(NCHK):
            src_chunk(c)
            for blk in range(PCHK[c][0] // 4, PCHK[c][1] // 4):
                pst = p2pool.tile([128, 128], f32r, name=f"pst{blk}", tag="ps2")
                nc.tensor.transpose(pst[:], src_t[:, blk * 128:(blk + 1) * 128],
                                    id_t[:])
                nc.vector.tensor_copy(ts_t[:, blk * 128:(blk + 1) * 128], pst[:])

        # --- phase 2: exponent matmuls + EXPs ---
        for blk in range(NB):
            rad = rpool.tile([128, 4 * WTOT], bf16, name=f"rad{blk}", tag="rad")
            rads[blk] = rad
            for half in range(2):
                psu = pupool.tile([128, 1024], f32, name=f"psu{blk}_{half}",
                                  tag="psu")
                for e in range(2):
                    p0 = 32 * (half * 2 + e)
                    nc.tensor.matmul(psu[:, e * SPC:e * SPC + WTOT],
                                     ts_t[p0:p0 + KR, blk * 128:(blk + 1) * 128],
                                     bd_t[p0:p0 + KR, :],
                                     start=True, stop=True,
                                     tile_position=(p0, 0))
                nc.scalar.activation(
                    rad[:].rearrange("p (e g) -> p e g", e=4)
                        [:, 2 * half:2 * half + 2, :],
                    psu[:].rearrange("p (e g) -> p e g", e=2)[:, :, 0:WTOT],
                    Act.Exp, bias=bias0[:])

        # --- phase 3: angular chain + accumulation matmuls per chunk ---
        for c in range(NCHK):
            ang_chunk(c)
            b0, b1 = PCHK[c][0] // 4, PCHK[c][1] // 4
            for blk in range(b0, b1):
                rad = rads[blk]
                for e in range(4):
                    pair = blk * 4 + e
                    t = pair // 16
                    q = pair % 16
                    pc = get_pc(t)
                    for j in range(nch):
                        g = gof[j]
                        nc.tensor.matmul(
                            pc[32 * g:32 * g + W0, q * F:(q + 1) * F],
                            rad[:, e * WTOT + j * W0:e * WTOT + (j + 1) * W0],
                            pall_s[:, pair * nch + j, :],
                            start=(j == gfirst[g]), stop=(j == glast[g]),
                            tile_position=(0, 32 * g),
                        )
                if blk % 4 == 3:
                    t = blk // 4
                    pc = _pc_tiles[t]
                    sb = gpool.tile([128, 128], f32r, name=f"sb{t}", tag="sb")
                    nc.vector.tensor_copy(sb[:], pc[:])
                    ps2 = p2pool.tile([R, 128], f32, name=f"ps2_{t}", tag="ps2")
                    nc.tensor.matmul(ps2[:], sel_t[:], sb[:], start=True, stop=True)
                    nc.vector.tensor_copy(outs_t[:, t * 128:(t + 1) * 128], ps2[:])
                    nc.sync.dma_start(d_out.ap()[:, t * 128:(t + 1) * 128],
                                      outs_t[:, t * 128:(t + 1) * 128])

    nc.compile()
    return nc


def _windows(rbar_sorted, nch):
    """Per-chunk offset windows -> 4 groups sharing an exact uniform window."""
    import itertools
    pos = (rbar_sorted - 0.5) / DELTA
    los, his = [], []
    for j in range(nch):
        blk = pos[:, j * 128:(j + 1) * 128]
        los.append(max(0, int(np.floor(blk.min() - RADI))))
        his.append(min(31, int(np.ceil(blk.max() + RADI))))
    best = None
    for splits in itertools.combinations(range(1, nch), 3):
        bnds = [0] + list(splits) + [nch]
        ws, ok, tot = [], True, 0
        for a, b in zip(bnds[:-1], bnds[1:]):
            lo, hi = min(los[a:b]), max(his[a:b])
            if hi - lo + 1 > 32:
                ok = False
                break
            ws.append((lo, hi))
            tot += (hi - lo + 1) * (b - a)
        if ok and (best is None or tot < best[0]):
            best = (tot, bnds, ws)
    if best is None:                      # fallback: full grid
        bnds, ws = [0, nch // 4, nch // 2, 3 * nch // 4, nch], [(0, 31)] * 4
    else:
        bnds, ws = best[1], best[2]
    W0 = max(hi - lo + 1 for lo, hi in ws)
    W0 += W0 % 2            # f32r matmul needs an even column count
    glo = [max(0, min(lo, 32 - W0)) for lo, hi in ws]
    gof = []
    for g in range(4):
        gof += [g] * (bnds[g + 1] - bnds[g])
    gfirst = [bnds[g] for g in range(4)]
    glast = [bnds[g + 1] - 1 for g in range(4)]
    return W0, tuple(glo), tuple(gof), tuple(gfirst), tuple(glast)


def _prep(r_ij, r_ik, r_jk, offsets, triple_masks):
    """Host-side shard + compact + sort-by-rbar + pad + transpose."""
    B, A, N = r_ij.shape
    P = B * A
    rij = np.ascontiguousarray(r_ij, dtype=np.float32).reshape(P, N)
    rik = np.ascontiguousarray(r_ik, dtype=np.float32).reshape(P, N)
    rjk = np.ascontiguousarray(r_jk, dtype=np.float32).reshape(P, N)
    m = (np.asarray(triple_masks).reshape(P, N) != 0)

    counts = m.sum(axis=1)
    npad = max(128, int(-(-max(1, counts.max()) // 128) * 128))
    nch = npad // 128

    cij = np.full((P, npad), 5.0, dtype=np.float32)
    cik = np.full((P, npad), 5.0, dtype=np.float32)
    cjk = np.full((P, npad), 5.0, dtype=np.float32)
    for p in range(P):
        idx = np.nonzero(m[p])[0]
        k = idx.size
        cij[p, :k] = rij[p, idx]
        cik[p, :k] = rik[p, idx]
        cjk[p, :k] = rjk[p, idx]

    order = np.argsort(cij + cik + cjk, axis=1, kind="stable")
    cij = np.take_along_axis(cij, order, 1)
    cik = np.take_along_axis(cik, order, 1)
    cjk = np.take_along_axis(cjk, order, 1)
    rbar = (cij + cik + cjk) / 3.0

    W0, glo, gof, gfirst, glast = _windows(rbar, nch)
    cfg = (nch, W0, glo, gof, gfirst, glast)

    o = np.asarray(offsets, dtype=np.float64)
    WTOT = nch * W0
    bd = np.zeros((128, WTOT), dtype=np.float32)
    for g4 in range(4):
        for j in range(nch):
            osl = o[glo[gof[j]]:glo[gof[j]] + W0]
            bd[32 * g4 + j, j * W0:(j + 1) * W0] = 2.0 * GAMMA * osl
            bd[32 * g4 + nch + j, j * W0:(j + 1) * W0] = -GAMMA
            bd[32 * g4 + 2 * nch, j * W0:(j + 1) * W0] = -3.0 * GAMMA * osl * osl
    sel = np.zeros((128, R), dtype=np.float32)
    for g4 in range(4):
        for rl in range(W0):
            sel[32 * g4 + rl, glo[g4] + rl] = 1.0

    def core_xpose(x, lo, hi):
        # chunk-major: X[p, pair*nch+j] = x[pair, j*128+p]
        return np.ascontiguousarray(
            x[lo:hi].reshape(PP, nch, 128).transpose(2, 0, 1).reshape(128, PP * nch))

    in_maps = []
    for c in range(N_CORES):
        lo, hi = c * PP, (c + 1) * PP
        in_maps.append({
            "rij": core_xpose(cij, lo, hi),
            "rik": core_xpose(cik, lo, hi),
            "rjk": core_xpose(cjk, lo, hi),
            "bdiag": bd, "sel": sel, "ident": np.eye(128, dtype=np.float32),
        })
    return in_maps, cfg


def _ensure_ntff_hook():
    """Register the axon NTFF profile hook if the image's antenv lacks it."""
    import types
    try:
        from antenv.axon_hooks import get_axon_ntff_profile_hook  # noqa: F401
        return
    except ImportError:
        pass
    try:
        sys.path.insert(0, "/root/.axon_site")
        from trn_agent_boot.trn_boot import _ntff_profile_via_ctypes
        hook = _ntff_profile_via_ctypes("/opt/axon/libaxon_pjrt.so")
        import antenv
        mod = types.ModuleType("antenv.axon_hooks")
        _holder = {"h": hook}
        mod.set_axon_ntff_profile_hook = lambda h: _holder.update(h=h)
        mod.get_axon_ntff_profile_hook = lambda: _holder["h"]
        sys.modules["antenv.axon_hooks"] = mod
        antenv.axon_hooks = mod
    except Exception:
        pass


def kernel(r_ij, r_ik, r_jk, offsets, triple_masks):
    global LAST_EXEC_NS
    from concourse.bass_utils import run_bass_kernel_spmd
    _ensure_ntff_hook()

    B, A, N = r_ij.shape
    in_maps, cfg = _prep(r_ij, r_ik, r_jk, offsets, triple_masks)
    if cfg not in _CACHE:
        _CACHE[cfg] = _build(cfg)
    nc = _CACHE[cfg]

    trace = os.environ.get("KERNEL_TRACE", "0") == "1"
    res = run_bass_kernel_spmd(nc, in_maps, core_ids=list(range(N_CORES)),
                               trace=trace)
    LAST_EXEC_NS = res.exec_time_ns
    outs = []
    for r in res.results:
        # [32, 512]: cols pair*8+f
        a = r["out"].reshape(R, PP, F).transpose(1, 0, 2).reshape(PP, R * F)
        outs.append(a)
    out = np.concatenate(outs, axis=0)
    return out.reshape(B, A, R * F)


# revision 36
# speedup vs baseline: 1.3898x; 1.1970x over previous
"""AngularDistribution Trainium2 kernel (8 NeuronCores, SPMD over (batch,atom) pairs).

Math (per pair p, triple n, offset r, filter f):
  rad[n,r]  = exp(-g*(S2[n] - 2*o_r*S1[n] + 3*o_r^2))   S1=rij+rik+rjk, S2=sum sq
  ang[n,f]  = 2*u^z (f<4, u=(1-ct)/2) or 2*v^z (f>=4), z in {1,2,4,8}
  cm[n]     = 2*(cos(pi*rij/10)*cos(pi*rik/10)*cos(pi*rjk/10))^2
  out[p,r*8+f] = sum_n rad[n,r] * pw[n,f] * cm[n]    (mask via host compaction)

v4 structure per core (64 pairs, nch*128 triples padded, sorted by rbar):
  - host compacts, SORTS each pair's triples by rbar=(S1/3), pre-transposes to
    [128, 64*nch] (chunk-major: chunk j = rbar quantile block), contiguous DMA
  - rad[n, r] is ~zero outside |o_r - rbar| < 0.65, so each chunk only needs a
    W0-wide offset window; chunks are grouped into 4 groups sharing an exact
    window [lo_g, lo_g+W0) -> exponent matmul streams nch*W0 cols (not nch*32)
  - tiny matmuls accumulate window-LOCAL into 4 aligned 32-row PSUM bands;
    a constant 0/1 matrix un-shifts bands to global r via one matmul per 16
    pairs
  - S1/S2 source tile transposed on the PE in f32r (half the f32 cost)
  - power/cutoff chain in bf16 (2x DVE modes); exponent path stays f32
"""

import os
import sys

sys.path.insert(0, "/opt/trn_rl_repo")

import numpy as np
from contextlib import ExitStack

GAMMA = 4.0
N_CORES = 8
PP = 64          # pairs per core (512 total / 8)
R = 32
F = 8
DELTA = 4.5 / 31  # offset grid spacing
RADI = 4.0        # window radius in grid units (= 0.58 distance)

_CACHE = {}
LAST_EXEC_NS = None


def _build(cfg):
    nch, W0, lo_of_chunk, gof, gfirst, glast, n9 = cfg
    import concourse.bass as bass
    import concourse.tile as tile
    from concourse import bacc, mybir

    f32 = mybir.dt.float32
    f32r = mybir.dt.float32r
    bf16 = mybir.dt.bfloat16
    Alu = mybir.AluOpType
    Act = mybir.ActivationFunctionType
    W = PP * nch          # global tile free size
    KR = 2 * nch + 1      # lhsT rows per pair (S1 rows, S2 rows, ones row)
    KRP = 32              # padded row stride: 4 pairs per 128-row block
    NV = PP * KRP
    NB = NV // 128        # 128-col blocks in src/ts (== PP//4)
    WTOT = nch * W0       # exponent grid cols per pair (windowed)
    SPC = 512             # psu column spacing per pair (bank-aligned)
    PI = float(np.pi)
    assert WTOT <= SPC

    nc = bacc.Bacc("TRN2", target_bir_lowering=False, debug=False,
                   num_devices=N_CORES)

    d_rij = nc.dram_tensor("rij", [128, W], f32, kind="ExternalInput")
    d_rik = nc.dram_tensor("rik", [128, W], f32, kind="ExternalInput")
    d_rjk = nc.dram_tensor("rjk", [128, W], f32, kind="ExternalInput")
    d_bd = nc.dram_tensor("bdiag", [128, WTOT], f32, kind="ExternalInput")
    d_sel = nc.dram_tensor("sel", [128, R], f32, kind="ExternalInput")
    d_id = nc.dram_tensor("ident", [128, 128], f32, kind="ExternalInput")
    d_out = nc.dram_tensor("out", [R, PP * F], f32, kind="ExternalOutput")

    with tile.TileContext(nc) as tc, ExitStack() as ctx:
        cpool = ctx.enter_context(tc.tile_pool(name="consts", bufs=1))
        gpool = ctx.enter_context(tc.tile_pool(name="glob", bufs=1))
        rpool = ctx.enter_context(tc.tile_pool(name="rad", bufs=16))
        pupool = ctx.enter_context(tc.tile_pool(name="psu", bufs=2, space="PSUM"))
        pcpool = ctx.enter_context(tc.tile_pool(name="pc", bufs=2, space="PSUM"))
        p2pool = ctx.enter_context(tc.tile_pool(name="ps2", bufs=2, space="PSUM"))

        bias0 = cpool.tile([128, 1], f32)
        nc.vector.memset(bias0[:], 0.0)
        bias_hpi = cpool.tile([128, 1], f32)
        nc.vector.memset(bias_hpi[:], PI / 2.0)

        # ---- input tiles, contiguous DMA in pair chunks (small first) ----
        rij_t = gpool.tile([128, W], f32)
        rik_t = gpool.tile([128, W], f32)
        rjk_t = gpool.tile([128, W], f32)
        PCHK = [(0, 8), (8, 32), (32, 64)]
        NCHK = len(PCHK)

        def csl(c):
            return slice(PCHK[c][0] * nch, PCHK[c][1] * nch)

        for c in range(NCHK):
            sl = csl(c)
            for dst, src in ((rij_t, d_rij), (rik_t, d_rik), (rjk_t, d_rjk)):
                nc.sync.dma_start(dst[:, sl], src.ap()[:, sl])
        bd_t = cpool.tile([128, WTOT], f32r)
        nc.sync.dma_start(bd_t[:], d_bd.ap().bitcast(f32r))
        sel_t = cpool.tile([128, R], f32r)
        nc.sync.dma_start(sel_t[:], d_sel.ap().bitcast(f32r))
        id_t = cpool.tile([128, 128], f32r)
        nc.sync.dma_start(id_t[:], d_id.ap().bitcast(f32r))

        # ---- working tiles (exponent path f32, angular path bf16) ----
        tij2 = gpool.tile([128, W], f32)
        tik2 = gpool.tile([128, W], f32)
        tjk2 = gpool.tile([128, W], f32)
        s12 = gpool.tile([128, W], f32)
        num = gpool.tile([128, W], f32)
        den = gpool.tile([128, W], f32)
        rden = gpool.tile([128, W], f32)
        hh = gpool.tile([128, W], f32)
        s1a = gpool.tile([128, W], f32)
        c1 = gpool.tile([128, W], bf16)
        c2 = gpool.tile([128, W], bf16)
        c3 = gpool.tile([128, W], bf16)
        p12 = gpool.tile([128, W], bf16)
        p2 = gpool.tile([128, W], bf16)
        cm = gpool.tile([128, W], bf16)
        u1 = gpool.tile([128, W], bf16)
        v1 = gpool.tile([128, W], bf16)
        u2 = gpool.tile([128, W], bf16)
        v2 = gpool.tile([128, W], bf16)
        u4 = gpool.tile([128, W], bf16)
        v4 = gpool.tile([128, W], bf16)
        u8 = gpool.tile([128, W], bf16)
        v8 = gpool.tile([128, W], bf16)
        pall = gpool.tile([128, F * W], bf16)
        pall_s = pall[:].rearrange("p (f col) -> p col f", f=F)

        src_t = gpool.tile([128, NV], f32r)
        src3 = src_t[:].rearrange("p (pair k) -> p pair k", k=KRP)
        nc.gpsimd.memset(src_t[:].bitcast(f32), 0.0)
        ones_t = gpool.tile([128, PP], f32)
        nc.vector.memset(ones_t[:], 1.0)
        nc.vector.tensor_copy(src3[:, :, 2 * nch:2 * nch + 1],
                              ones_t[:].rearrange("p (pair k) -> p pair k", k=1))
        ts_t = gpool.tile([128, NV], f32r)
        s1_3 = src3[:, :, 0:nch]
        s2_3 = src3[:, :, nch:2 * nch]

        outs_t = gpool.tile([R, PP * F], f32)

        # ---- scalar transcendentals for ALL chunks first (2 table loads) ----
        for c in range(NCHK):
            sl = csl(c)
            nc.scalar.activation(c1[:, sl], rij_t[:, sl], Act.Sin,
                                 scale=PI / 10.0, bias=bias_hpi[:])
            nc.scalar.activation(c2[:, sl], rik_t[:, sl], Act.Sin,
                                 scale=PI / 10.0, bias=bias_hpi[:])
            nc.scalar.activation(c3[:, sl], rjk_t[:, sl], Act.Sin,
                                 scale=PI / 10.0, bias=bias_hpi[:])

        def src_chunk(c):
            # short path feeding the exponent matmuls: squares + sums only
            sl = csl(c)
            pr = slice(PCHK[c][0], PCHK[c][1])
            nc.gpsimd.tensor_tensor(tij2[:, sl], rij_t[:, sl], rij_t[:, sl], Alu.mult)
            nc.gpsimd.tensor_tensor(tik2[:, sl], rik_t[:, sl], rik_t[:, sl], Alu.mult)
            nc.vector.tensor_tensor(tjk2[:, sl], rjk_t[:, sl], rjk_t[:, sl], Alu.mult)
            nc.vector.tensor_tensor(s12[:, sl], tij2[:, sl], tik2[:, sl], Alu.add)
            nc.gpsimd.tensor_tensor(s1a[:, sl], rij_t[:, sl], rik_t[:, sl], Alu.add)
            nc.gpsimd.tensor_tensor(
                s1_3[:, pr, :],
                s1a[:, sl].rearrange("p (pair j) -> p pair j", j=nch),
                rjk_t[:, sl].rearrange("p (pair j) -> p pair j", j=nch), Alu.add)
            nc.vector.tensor_tensor(
                s2_3[:, pr, :],
                s12[:, sl].rearrange("p (pair j) -> p pair j", j=nch),
                tjk2[:, sl].rearrange("p (pair j) -> p pair j", j=nch), Alu.add)

        def ang_chunk(c):
            # angular/cutoff chain: bf16 ops mostly on Vector (2x modes),
            # f32 ops mostly on GpSimd (dtype-blind)
            sl = csl(c)
            nc.gpsimd.tensor_tensor(den[:, sl], rij_t[:, sl], rik_t[:, sl], Alu.mult)
            nc.vector.reciprocal_approx_fast(rden[:, sl], den[:, sl])
            nc.vector.scalar_tensor_tensor(num[:, sl], tjk2[:, sl], -1.0, s12[:, sl],
                                           Alu.mult, Alu.add)
            nc.vector.scalar_tensor_tensor(hh[:, sl], num[:, sl], -0.25, rden[:, sl],
                                           Alu.mult, Alu.mult)
            nc.vector.tensor_scalar(u1[:, sl], hh[:, sl], 0.5, None, Alu.add)
            nc.vector.tensor_scalar(v1[:, sl], hh[:, sl], -1.0, 0.5,
                                    Alu.mult, Alu.add)
            nc.gpsimd.tensor_tensor(p12[:, sl], c1[:, sl], c2[:, sl], Alu.mult)
            nc.gpsimd.tensor_tensor(p2[:, sl], p12[:, sl], c3[:, sl], Alu.mult)
            nc.vector.scalar_tensor_tensor(cm[:, sl], p2[:, sl], 2.0, p2[:, sl],
                                           Alu.mult, Alu.mult)
            nc.gpsimd.tensor_tensor(u2[:, sl], u1[:, sl], u1[:, sl], Alu.mult)
            nc.vector.tensor_tensor(v2[:, sl], v1[:, sl], v1[:, sl], Alu.mult)
            nc.gpsimd.tensor_tensor(u4[:, sl], u2[:, sl], u2[:, sl], Alu.mult)
            nc.vector.tensor_tensor(v4[:, sl], v2[:, sl], v2[:, sl], Alu.mult)
            nc.gpsimd.tensor_tensor(u8[:, sl], u4[:, sl], u4[:, sl], Alu.mult)
            nc.vector.tensor_tensor(v8[:, sl], v4[:, sl], v4[:, sl], Alu.mult)
            lo, hi = PCHK[c][0] * nch, PCHK[c][1] * nch
            for fi, pw in enumerate((u1, u2, u4, u8, v1, v2, v4, v8)):
                eng = nc.vector if fi % 2 == 0 else nc.gpsimd
                eng.tensor_tensor(pall[:, fi * W + lo:fi * W + hi],
                                  pw[:, sl], cm[:, sl], Alu.mult)

        _pc_tiles = {}

        def get_pc(t):
            if t not in _pc_tiles:
                _pc_tiles[t] = pcpool.tile([128, 128], f32, name=f"pc{t}", tag="pc")
                nc.vector.memset(_pc_tiles[t][:], 0.0)
            return _pc_tiles[t]

        # --- phase 1: per chunk: src tiles + transposes (short dep path) ---
        rads = [None] * NB
        for c in range(NCHK):
            src_chunk(c)
            for blk in range(PCHK[c][0] // 4, PCHK[c][1] // 4):
                pst = p2pool.tile([128, 128], f32r, name=f"pst{blk}", tag="ps2")
                nc.tensor.transpose(pst[:], src_t[:, blk * 128:(blk + 1) * 128],
                                    id_t[:])
                nc.vector.tensor_copy(ts_t[:, blk * 128:(blk + 1) * 128], pst[:])

        # --- phase 2: exponent matmuls + EXPs ---
        for blk in range(NB):
            rad = rpool.tile([128, 4 * WTOT], bf16, name=f"rad{blk}", tag="rad")
            rads[blk] = rad
            for half in range(2):
                psu = pupool.tile([128, 1024], f32, name=f"psu{blk}_{half}",
                                  tag="psu")
                for e in range(2):
                    p0 = 32 * (half * 2 + e)
                    nc.tensor.matmul(psu[:, e * SPC:e * SPC + WTOT],
                                     ts_t[p0:p0 + KR, blk * 128:(blk + 1) * 128],
                                     bd_t[p0:p0 + KR, :],
                                     start=True, stop=True,
                                     tile_position=(p0, 0))
                nc.scalar.activation(
                    rad[:].rearrange("p (e g) -> p e g", e=4)
                        [:, 2 * half:2 * half + 2, :],
                    psu[:].rearrange("p (e g) -> p e g", e=2)[:, :, 0:WTOT],
                    Act.Exp, bias=bias0[:])

        # --- phase 3: angular chain + accumulation matmuls per chunk ---
        for c in range(NCHK):
            ang_chunk(c)
            b0, b1 = PCHK[c][0] // 4, PCHK[c][1] // 4
            for blk in range(b0, b1):
                rad = rads[blk]
                for e in range(4):
                    pair = blk * 4 + e
                    t = pair // 16
                    q = pair % 16
                    pc = get_pc(t)
                    nw = nch if pair < n9 else nch - 1
                    for j in range(nw):
                        g = gof[j]
                        nc.tensor.matmul(
                            pc[32 * g:32 * g + W0, q * F:(q + 1) * F],
                            rad[:, e * WTOT + j * W0:e * WTOT + (j + 1) * W0],
                            pall_s[:, pair * nch + j, :],
                            start=(j == gfirst[g]),
                            stop=(j == glast[g] or j == nw - 1),
                            tile_position=(0, 32 * g),
                        )
                if blk % 4 == 3:
                    t = blk // 4
                    pc = _pc_tiles[t]
                    sb = gpool.tile([128, 128], f32r, name=f"sb{t}", tag="sb")
                    nc.vector.tensor_copy(sb[:], pc[:])
                    ps2 = p2pool.tile([R, 128], f32, name=f"ps2_{t}", tag="ps2")
                    nc.tensor.matmul(ps2[:], sel_t[:], sb[:], start=True, stop=True)
                    nc.vector.tensor_copy(outs_t[:, t * 128:(t + 1) * 128], ps2[:])
                    nc.sync.dma_start(d_out.ap()[:, t * 128:(t + 1) * 128],
                                      outs_t[:, t * 128:(t + 1) * 128])

    nc.compile()
    return nc


def _windows(rbar_sorted, nch):
    """Per-chunk offset windows -> 4 groups sharing an exact uniform window."""
    import itertools
    pos = (rbar_sorted - 0.5) / DELTA
    los, his = [], []
    for j in range(nch):
        blk = pos[:, j * 128:(j + 1) * 128]
        los.append(max(0, int(np.floor(blk.min() - RADI))))
        his.append(min(31, int(np.ceil(blk.max() + RADI))))
    best = None
    for splits in itertools.combinations(range(1, nch), 3):
        bnds = [0] + list(splits) + [nch]
        ws, ok, tot = [], True, 0
        for a, b in zip(bnds[:-1], bnds[1:]):
            lo, hi = min(los[a:b]), max(his[a:b])
            if hi - lo + 1 > 32:
                ok = False
                break
            ws.append((lo, hi))
            tot += (hi - lo + 1) * (b - a)
        if ok and (best is None or tot < best[0]):
            best = (tot, bnds, ws)
    if best is None:                      # fallback: full grid
        bnds, ws = [0, nch // 4, nch // 2, 3 * nch // 4, nch], [(0, 31)] * 4
    else:
        bnds, ws = best[1], best[2]
    W0 = max(hi - lo + 1 for lo, hi in ws)
    W0 += W0 % 2            # f32r matmul needs an even column count
    glo = [max(0, min(lo, 32 - W0)) for lo, hi in ws]
    gof = []
    for g in range(4):
        gof += [g] * (bnds[g + 1] - bnds[g])
    gfirst = [bnds[g] for g in range(4)]
    glast = [bnds[g + 1] - 1 for g in range(4)]
    return W0, tuple(glo), tuple(gof), tuple(gfirst), tuple(glast)


def _prep(r_ij, r_ik, r_jk, offsets, triple_masks):
    """Host-side shard + compact + sort-by-rbar + pad + transpose."""
    B, A, N = r_ij.shape
    P = B * A
    rij = np.ascontiguousarray(r_ij, dtype=np.float32).reshape(P, N)
    rik = np.ascontiguousarray(r_ik, dtype=np.float32).reshape(P, N)
    rjk = np.ascontiguousarray(r_jk, dtype=np.float32).reshape(P, N)
    m = (np.asarray(triple_masks).reshape(P, N) != 0)

    counts = m.sum(axis=1)
    npad = max(128, int(-(-max(1, counts.max()) // 128) * 128))
    nch = npad // 128

    cij = np.full((P, npad), 5.0, dtype=np.float32)
    cik = np.full((P, npad), 5.0, dtype=np.float32)
    cjk = np.full((P, npad), 5.0, dtype=np.float32)
    for p in range(P):
        idx = np.nonzero(m[p])[0]
        k = idx.size
        cij[p, :k] = rij[p, idx]
        cik[p, :k] = rik[p, idx]
        cjk[p, :k] = rjk[p, idx]

    order = np.argsort(cij + cik + cjk, axis=1, kind="stable")
    cij = np.take_along_axis(cij, order, 1)
    cik = np.take_along_axis(cik, order, 1)
    cjk = np.take_along_axis(cjk, order, 1)

    # per-core: big-count pairs first so short pairs can skip their last chunk
    perms = []
    n9 = 0
    for c in range(N_CORES):
        lo, hi = c * PP, (c + 1) * PP
        perm = np.argsort(-counts[lo:hi], kind="stable")
        perms.append(perm)
        cij[lo:hi] = cij[lo:hi][perm]
        cik[lo:hi] = cik[lo:hi][perm]
        cjk[lo:hi] = cjk[lo:hi][perm]
        n9 = max(n9, int((counts[lo:hi][perm] > (nch - 1) * 128).sum()))
    rbar = (cij + cik + cjk) / 3.0

    W0, glo, gof, gfirst, glast = _windows(rbar, nch)
    cfg = (nch, W0, glo, gof, gfirst, glast, n9)

    o = np.asarray(offsets, dtype=np.float64)
    WTOT = nch * W0
    bd = np.zeros((128, WTOT), dtype=np.float32)
    for g4 in range(4):
        for j in range(nch):
            osl = o[glo[gof[j]]:glo[gof[j]] + W0]
            bd[32 * g4 + j, j * W0:(j + 1) * W0] = 2.0 * GAMMA * osl
            bd[32 * g4 + nch + j, j * W0:(j + 1) * W0] = -GAMMA
            bd[32 * g4 + 2 * nch, j * W0:(j + 1) * W0] = -3.0 * GAMMA * osl * osl
    sel = np.zeros((128, R), dtype=np.float32)
    for g4 in range(4):
        for rl in range(W0):
            sel[32 * g4 + rl, glo[g4] + rl] = 1.0

    def core_xpose(x, lo, hi):
        # chunk-major: X[p, pair*nch+j] = x[pair, j*128+p]
        return np.ascontiguousarray(
            x[lo:hi].reshape(PP, nch, 128).transpose(2, 0, 1).reshape(128, PP * nch))

    in_maps = []
    for c in range(N_CORES):
        lo, hi = c * PP, (c + 1) * PP
        in_maps.append({
            "rij": core_xpose(cij, lo, hi),
            "rik": core_xpose(cik, lo, hi),
            "rjk": core_xpose(cjk, lo, hi),
            "bdiag": bd, "sel": sel, "ident": np.eye(128, dtype=np.float32),
        })
    return in_maps, cfg, perms


def _ensure_ntff_hook():
    """Register the axon NTFF profile hook if the image's antenv lacks it."""
    import types
    try:
        from antenv.axon_hooks import get_axon_ntff_profile_hook  # noqa: F401
        return
    except ImportError:
        pass
    try:
        sys.path.insert(0, "/root/.axon_site")
        from trn_agent_boot.trn_boot import _ntff_profile_via_ctypes
        hook = _ntff_profile_via_ctypes("/opt/axon/libaxon_pjrt.so")
        import antenv
        mod = types.ModuleType("antenv.axon_hooks")
        _holder = {"h": hook}
        mod.set_axon_ntff_profile_hook = lambda h: _holder.update(h=h)
        mod.get_axon_ntff_profile_hook = lambda: _holder["h"]
        sys.modules["antenv.axon_hooks"] = mod
        antenv.axon_hooks = mod
    except Exception:
        pass


def kernel(r_ij, r_ik, r_jk, offsets, triple_masks):
    global LAST_EXEC_NS
    from concourse.bass_utils import run_bass_kernel_spmd
    _ensure_ntff_hook()

    B, A, N = r_ij.shape
    in_maps, cfg, perms = _prep(r_ij, r_ik, r_jk, offsets, triple_masks)
    if cfg not in _CACHE:
        _CACHE[cfg] = _build(cfg)
    nc = _CACHE[cfg]

    trace = os.environ.get("KERNEL_TRACE", "0") == "1"
    res = run_bass_kernel_spmd(nc, in_maps, core_ids=list(range(N_CORES)),
                               trace=trace)
    LAST_EXEC_NS = res.exec_time_ns
    outs = []
    for c, r in enumerate(res.results):
        # [32, 512]: cols slot*8+f; un-permute slots back to original pairs
        a = r["out"].reshape(R, PP, F).transpose(1, 0, 2).reshape(PP, R * F)
        ao = np.empty_like(a)
        ao[perms[c]] = a
        outs.append(ao)
    out = np.concatenate(outs, axis=0)
    return out.reshape(B, A, R * F)
